# revision 21
# baseline (speedup 1.0000x reference)
"""Equivariant attention (gnn_message_passing) on 8 Trainium2 NeuronCores.

Strategy (head-sharded tensor parallel, core c owns head c):

The reference materializes [H, N, E] scores/attn over E=8192 edges. Here the
edge dimension is collapsed onto the N=512 atoms at projection level:

  scores[h, n, e]   = sf[h, n, a_e] + bias[h, edge_map[e]]     (a_e = atom_index)
  attn-softmax per (batch-segment, n) then  out = attn @ vh_edges

factors exactly into dense [N, N] algebra with two tiny per-(segment, atom)
tables (NSEG=16 x N=512):

  D[g, m] = sum_{e in seg g, a_e = m} env_e   * exp(b_e)
  C[g, m] = sum_{e in seg g, a_e = m} env_e^2 * exp(b_e)
  den[g, n]  = sum_m exp(sf[m, n]) * D[g, m]           (one matmul)
  Aagg[m, n] = exp(sf[m, n]) * sum_g C[g, m] / den[g, n]
  out[n, f]  = Aagg^T @ vh[m, f]                        (one matmul)

The running-max subtraction in the reference softmax cancels exactly (up to a
+1e-16 epsilon whose relative effect is ~1e-16) and |scale*sf + b| < 20, so
unnormalized exp is safe in f32.

D/C are built on-device from "slot tensors": host packs per-(atom, segment)
edge lists into a fixed-width [128, 4*16*L2] layout (env & bias values; pads
have env=0 so they vanish), and a single free-axis reduce per table produces
it. Only integer index bookkeeping and value re-layout happen on host.

q/k/v arrive HOST-PRE-TRANSPOSED as qT/kT/vT [CIN, S*N] (channel-major), so
the kernel needs no on-device input transposes: projections read qT slices
directly.  DMAs are issued in first-use order so the PE starts ~1.5us in.

Phase 2 (per-core 64-atom slice): LN + output projection in yT [ci, (s, n)]
orientation.  The per-atom LN scaling is fused into the PE transpose by
replacing the identity operand with diag(scale) (the mean subtraction rides
along as a rank-1 ones x (-mu*rstd) matmul); gamma is folded into the output
weights on device, beta/bo become per-partition biases of the PSUM extract.
Host un-transposes the yT output for free.

Both phases are bf16 on the big operands (inputs, weights, intermediates
bound for matmuls); f32 is kept for the softmax denominators, LN statistics
and the final output.  HW-verified rel err ~8e-3 (gate 2e-2).
"""

import os
import numpy as np

import concourse.bass as bass
import concourse.tile as tile
from concourse import mybir
from concourse.bass_utils import run_bass_kernel_spmd
from concourse.masks import make_identity

# ---------------------------------------------------------------- constants
H, LMAX, NSEG = 8, 2, 16
S = (LMAX + 1) ** 2          # 9 spherical components
N, E, CIN, CH = 512, 8192, 128, 256
D = CH // H                  # 32 per-head channels
F = S * D                    # 288 per-head feature width
NT = N // 128                # 4 atom tiles
NR = N // H                  # 64 atoms per core in the LN/out stage
EPS = 1e-7
SCALE = float(np.sqrt(D / 3.0) / D)
L_OF_M = np.floor(np.sqrt(np.arange(S))).astype(np.int64)
F32 = mybir.dt.float32
F32R = mybir.dt.float32r
BF16 = mybir.dt.bfloat16
AF = mybir.ActivationFunctionType
ALU = mybir.AluOpType

import ml_dtypes
NP_BF16 = ml_dtypes.bfloat16

_DBG = bool(int(os.environ.get("KBDBG", "0")))


def _split_multiwaits(nc: bass.Bass, limit: int = 1):
    """This walrus build rejects instructions carrying more than one semaphore
    wait (and Drains carrying any). Hoist excess waits onto NOPs inserted just
    before the instruction on the same engine - semantically identical."""
    for f in nc.m.functions:
        for blk in f.blocks:
            changed = False
            out = []
            for inst in blk.instructions:
                si = inst.sync_info
                waits = list(si.on_wait) if si is not None else []
                keep = 0 if inst.opcode == "Drain" else limit
                if len(waits) > keep:
                    hoist = waits[: len(waits) - keep]
                    rest = waits[len(waits) - keep:]
                    for w in hoist:
                        nop = mybir.InstNoOp(
                            name=f"{inst.name}-w{len(out)}", ins=[], outs=[]
                        )
                        nop.engine = inst.engine
                        nop.sync_info = mybir.SyncInfo(on_wait=[w], on_update=[])
                        out.append(nop)
                    inst.sync_info = mybir.SyncInfo(
                        on_wait=rest, on_update=list(si.on_update)
                    )
                    changed = True
                out.append(inst)
            if changed:
                blk.instructions = out


def build_bass(L2: int, loop_R: int | None = None) -> bass.Bass:
    """One SPMD program; per-core data (weight slices, bias slots) comes in as
    inputs. L2 = slot width per (atom, segment) cell."""
    W = NT * NSEG * L2  # slot tensor free width per partition

    nc = bass.Bass("TRN2", target_bir_lowering=False, debug=False, num_devices=H)

    # ------------------------------------------------------------- tensors
    # host-pre-transposed bf16 inputs: qT/kT [i, (s, m)]; vT mt-major
    # [i, (t, s, j)] so each quarter is a contiguous DMA
    qT_d = nc.dram_tensor("qT", [CIN, S * N], BF16, kind="ExternalInput")
    kT_d = nc.dram_tensor("kT", [CIN, S * N], BF16, kind="ExternalInput")
    vT_d = nc.dram_tensor("vT", [CIN, S * N], BF16, kind="ExternalInput")
    wq_d = nc.dram_tensor("wq", [CIN, S * D], BF16, kind="ExternalInput")  # [i,(s,o)]
    wk_d = nc.dram_tensor("wk", [CIN, S * D], BF16, kind="ExternalInput")
    wv_d = nc.dram_tensor("wv", [CIN, S * D], BF16, kind="ExternalInput")
    bqkv_d = nc.dram_tensor("bqkv", [D, 3], F32, kind="ExternalInput")
    bvrow_d = nc.dram_tensor("bvrow", [1, D], F32, kind="ExternalInput")
    envs_d = nc.dram_tensor("envs", [128, W], BF16, kind="ExternalInput")
    bs_d = nc.dram_tensor("bs", [128, W], BF16, kind="ExternalInput")
    ao_d = nc.dram_tensor("ao", [N, F], BF16, kind="ExternalOutput")

    with tile.TileContext(nc) as tc:
        with (
            tc.tile_pool(name="const", bufs=1) as cpool,
            tc.tile_pool(name="feat", bufs=1) as featp,
            tc.tile_pool(name="work", bufs=1) as workp,
            tc.tile_pool(name="aop", bufs=1) as aop,
            tc.tile_pool(name="ps", bufs=4, space="PSUM") as psp,
            tc.tile_pool(name="pop", bufs=1, space="PSUM") as pop,
        ):
            def ps_tile(shape):
                return psp.tile(shape, F32, tag="ps", name="ps")

            # -------- DMA issue, first-use order, spread over engines ------
            # sync queue:   wq, qT thirds, vT quarters (mt-major layout)
            # scalar queue: wk, wv, kT thirds, envs, bs
            # gpsimd queue: tiny constants
            TH = 3 * N
            wq_sb = cpool.tile([CIN, S * D], BF16, tag="wq", name="wq")
            wk_sb = cpool.tile([CIN, S * D], BF16, tag="wk", name="wk")
            wv_sb = cpool.tile([CIN, S * D], BF16, tag="wv", name="wv")
            qTc = [cpool.tile([CIN, TH], BF16, tag=f"qT{t}", name=f"qT{t}")
                   for t in range(3)]
            kTc = [cpool.tile([CIN, TH], BF16, tag=f"kT{t}", name=f"kT{t}")
                   for t in range(3)]
            # vT quarter mt holds [i, (s, m_local)] for atom tile mt
            vTq = [cpool.tile([CIN, S * 128], BF16, tag=f"vT{m}", name=f"vT{m}")
                   for m in range(NT)]
            envs_sb = cpool.tile([128, W], BF16, tag="envs", name="envs")
            bs_sb = cpool.tile([128, W], BF16, tag="bs", name="bs")

            def vtq_src(m):
                return vT_d[:].rearrange("i (t x) -> i t x", t=NT)[:, m, :]

            nc.sync.dma_start(wq_sb[:], wq_d[:])
            for t in range(3):
                nc.sync.dma_start(qTc[t][:], qT_d[:, t * TH:(t + 1) * TH])
            nc.sync.dma_start(vTq[0][:], vtq_src(0))
            nc.sync.dma_start(vTq[1][:], vtq_src(1))
            nc.scalar.dma_start(wk_sb[:], wk_d[:])
            for t in range(3):
                nc.scalar.dma_start(kTc[t][:], kT_d[:, t * TH:(t + 1) * TH])
            nc.scalar.dma_start(wv_sb[:], wv_d[:])
            nc.gpsimd.dma_start(envs_sb[:], envs_d[:])
            nc.gpsimd.dma_start(bs_sb[:], bs_d[:])
            nc.scalar.dma_start(vTq[2][:], vtq_src(2))
            nc.scalar.dma_start(vTq[3][:], vtq_src(3))
            bqkv_sb = cpool.tile([D, 3], F32, tag="bqkv", name="bqkv")
            nc.gpsimd.dma_start(bqkv_sb[:], bqkv_d[:])
            bvrow_sb = cpool.tile([128, D], F32, tag="bvrow", name="bvrow")
            nc.gpsimd.dma_start(bvrow_sb[:], bvrow_d[0:1, :].to_broadcast([128, D]))
            ident = cpool.tile([128, 128], F32, tag="ident", name="ident")
            make_identity(nc, ident[:])
            eps16 = cpool.tile([1, NSEG], F32, tag="eps16", name="eps16")
            nc.gpsimd.memset(eps16[:], 1e-16)
            ones_n = cpool.tile([1, N], F32, tag="ones_n", name="ones_n")
            nc.gpsimd.memset(ones_n[:], 1.0)

            import contextlib as _ctl
            _loop = tc.For_i(0, loop_R, 1) if loop_R else _ctl.nullcontext()
            with _loop:
                copy_engines = [nc.scalar, nc.vector]
                cp_i = 0

                def copy_alt(dst_ap, src_ap):
                    nonlocal cp_i
                    eng = copy_engines[cp_i % 2]
                    cp_i += 1
                    if eng is nc.scalar:
                        eng.copy(dst_ap, src_ap)
                    else:
                        eng.tensor_copy(out=dst_ap, in_=src_ap)

                def copy_dve(dst_ap, src_ap, eng=None):
                    if eng is nc.scalar:
                        eng.copy(dst_ap, src_ap)
                    else:
                        nc.vector.tensor_copy(out=dst_ap, in_=src_ap)

                # ------------------------------ D / C tables (early, off-path)
                ebs = workp.tile([128, W], F32, tag="ebs", name="ebs")
                nc.scalar.activation(ebs[:], bs_sb[:], AF.Exp)
                wD = workp.tile([128, W], F32, tag="wD", name="wD")
                nc.vector.tensor_tensor(out=wD[:], in0=envs_sb[:], in1=ebs[:], op=ALU.mult)
                wC = workp.tile([128, W], F32, tag="wC", name="wC")
                nc.vector.tensor_tensor(out=wC[:], in0=wD[:], in1=envs_sb[:], op=ALU.mult)
                d_t = featp.tile([128, NT * NSEG], F32, tag="d_t", name="d_t")  # [m_p, (mt, g)]
                c_t = featp.tile([128, NT * NSEG], F32, tag="c_t", name="c_t")
                with nc.allow_low_precision(reason="f32r is 32-bit storage"):
                    nc.vector.reduce_sum(
                        out=d_t[:].rearrange("p (t g) -> p t g", t=NT).bitcast(F32R),
                        in_=wD[:].rearrange("p (t g j) -> p t g j", t=NT, g=NSEG),
                        axis=mybir.AxisListType.X,
                    )
                nc.vector.reduce_sum(
                    out=c_t[:].rearrange("p (t g) -> p t g", t=NT),
                    in_=wC[:].rearrange("p (t g j) -> p t g j", t=NT, g=NSEG),
                    axis=mybir.AxisListType.X,
                )
                # ------- fq / fk chunk-streamed projection + psf accumulation
                # chunk layout: rows (s_local*32+o), chunks s=0..2 / 3..5 / 6..8
                # (96 rows per chunk so matmul outs land at base 0/32/64);
                # psf[mt] accumulates across chunks in 4 held banks (tags
                # shared with the po accumulators, which start strictly later)
                fq = [featp.tile([96, N], BF16, tag=f"fq{c}", name=f"fq{c}")
                      for c in range(3)]
                fk = [featp.tile([96, N], BF16, tag=f"fk{c}", name=f"fk{c}")
                      for c in range(3)]
                psf = [pop.tile([128, N], F32, tag=f"acc{mt}", name=f"psf{mt}")
                       for mt in range(NT)]
                # within chunk 0 the s components sit in row order (1, 2, 0)
                # so the biased s=0 rows are 64-aligned for the PSUM read
                # (fk uses the same permutation, so scores are unchanged)
                ROWOF = {0: 2, 1: 0, 2: 1}
                for chunk in range(3):
                    for t_c, w_sb, f_dst, t_idx in ((qTc, wq_sb, fq, 0),
                                                    (kTc, wk_sb, fk, 1)):
                        pp = ps_tile([96, N])
                        for j in range(3):
                            s = chunk * 3 + j
                            r = ROWOF[j] if chunk == 0 else j
                            nc.tensor.matmul(
                                pp[r * D:(r + 1) * D, :],
                                lhsT=w_sb[:, s * D:(s + 1) * D],
                                rhs=t_c[chunk][:, j * N:(j + 1) * N],
                                start=True, stop=True,
                            )
                        ceng = nc.scalar if t_idx == 0 else nc.vector
                        if chunk == 0:
                            # bias on s=0 rows (l=0 invariant component)
                            copy_dve(f_dst[0][0:64, :], pp[0:64, :], ceng)
                            nc.vector.tensor_scalar_add(
                                f_dst[0][64:96, :], pp[64:96, :],
                                bqkv_sb[:, t_idx:t_idx + 1])
                        else:
                            copy_dve(f_dst[chunk][:], pp[:], ceng)
                    for mt in range(NT):
                        nc.tensor.matmul(
                            psf[mt][:],
                            lhsT=fk[chunk][:, mt * 128:(mt + 1) * 128],
                            rhs=fq[chunk][:],
                            start=(chunk == 0), stop=(chunk == 2),
                            skip_group_check=True,
                        )

                # C transposed to [g, m]
                c_sb = featp.tile([NSEG, N], F32, tag="c_sb", name="c_sb")
                for mt in range(NT):
                    pc = ps_tile([NSEG, 128])
                    nc.tensor.transpose(
                        pc[:], c_t[:, mt * NSEG:(mt + 1) * NSEG], ident[:]
                    )
                    nc.vector.tensor_copy(out=c_sb[:, mt * 128:(mt + 1) * 128].bitcast(F32R), in_=pc[:])

                # -------------------------------- vhn [m, (s,o)] per m-tile
                vhn = [featp.tile([128, F], BF16, tag=f"vhn{mt}", name=f"vhn{mt}") for mt in range(NT)]
                for mt in range(NT):
                    pv = ps_tile([128, F])
                    for s in range(S):
                        nc.tensor.matmul(
                            pv[:, s * D:(s + 1) * D],
                            lhsT=vTq[mt][:, s * 128:(s + 1) * 128],
                            rhs=wv_sb[:, s * D:(s + 1) * D],
                            start=True, stop=True,
                        )
                    nc.vector.tensor_copy(out=vhn[mt][:, D:F], in_=pv[:, D:F])
                    nc.vector.tensor_tensor(
                        out=vhn[mt][:, 0:D],
                        in0=pv[:, 0:D], in1=bvrow_sb[:], op=ALU.add,
                    )

                # ------- exp + denominator, split in n-halves (h-major) so
                # the dd -> aggt -> att-out chain pipelines per half
                HN = N // 2
                exp_sf = [featp.tile([128, N], F32, tag=f"esf{mt}",
                                     name=f"esf{mt}") for mt in range(NT)]
                pden = [ps_tile([NSEG, HN]) for h in range(2)]
                for h in range(2):
                    nc.tensor.matmul(
                        pden[h][:], lhsT=eps16[:].bitcast(F32R),
                        rhs=ones_n[:, 0:HN].bitcast(F32R), start=True, stop=False,
                        skip_group_check=True,
                    )
                for mt in range(NT):
                    nc.scalar.activation(exp_sf[mt][:].bitcast(F32R), psf[mt][:],
                                         AF.Exp, scale=SCALE)
                    for h in range(2):
                        nc.tensor.matmul(
                            pden[h][:],
                            lhsT=d_t[:, mt * NSEG:(mt + 1) * NSEG].bitcast(F32R),
                            rhs=exp_sf[mt][:, h * HN:(h + 1) * HN].bitcast(F32R),
                            start=False, stop=(mt == NT - 1),
                            skip_group_check=True,
                        )

                # ------- per half: dd; per m-tile: aggt mult; att-out
                aggt = [[featp.tile([128, HN], BF16, tag=f"agg{mt}h{h}",
                                    name=f"agg{mt}h{h}") for h in range(2)]
                        for mt in range(NT)]
                po = [pop.tile([128, F], F32, tag=f"acc{nt}", name=f"po{nt}")
                      for nt in range(NT)]
                dd = [featp.tile([NSEG, HN], F32, tag=f"ddh{h}", name=f"ddh{h}")
                      for h in range(2)]
                for h in range(2):
                    with nc.allow_low_precision(reason="f32r is 32-bit storage"):
                        nc.vector.reciprocal(dd[h][:].bitcast(F32R), pden[h][:])
                    for mt in range(NT):
                        pT = ps_tile([128, HN])
                        nc.tensor.matmul(
                            pT[:],
                            lhsT=c_sb[:, mt * 128:(mt + 1) * 128].bitcast(F32R),
                            rhs=dd[h][:].bitcast(F32R),
                            start=True, stop=True,
                        )
                        nc.vector.tensor_tensor(
                            out=aggt[mt][h][:],
                            in0=exp_sf[mt][:, h * HN:(h + 1) * HN],
                            in1=pT[:], op=ALU.mult)
                        for j in range(2):
                            nt = 2 * h + j
                            nc.tensor.matmul(
                                po[nt][:],
                                lhsT=aggt[mt][h][:, j * 128:(j + 1) * 128],
                                rhs=vhn[mt][:],
                                start=(mt == 0), stop=(mt == NT - 1),
                                skip_group_check=True,
                            )
                # two ao DMAs; slice the t axis AFTER the rearrange (the
                # slice-then-rearrange form writes wrong DRAM locations)
                for pair in range(2):
                    ao = aop.tile([128, 2 * F], BF16, tag=f"ao{pair}",
                                  name=f"ao{pair}")
                    for j in range(2):
                        copy_alt(ao[:, j * F:(j + 1) * F], po[2 * pair + j][:])
                    nc.sync.dma_start(
                        ao_d[:].rearrange("(t p) f -> p t f", t=NT)
                        [:, 2 * pair:2 * pair + 2, :],
                        ao[:].rearrange("p (t f) -> p t f", t=2))

    _split_multiwaits(nc)
    return nc


SN = S * NR          # 576 free columns in phase 2, s-permuted (l=1, l=2, l=0)
SPERM = [1, 2, 3, 4, 5, 6, 7, 8, 0]   # column group j -> spherical component s
NL1 = 3 * NR         # 192: end of the l=1 block
NL2 = 8 * NR         # 512: end of the l=2 block (bank boundary)


def build_phase2(loop_R: int | None = None, debug_taps: bool = False) -> bass.Bass:
    """Equivariant layernorm + output projection on a 64-atom slice.

    The LN scaling is a per-(atom, l) scalar, so it commutes with the output
    projection: project the RAW (un-normalized, gamma-folded) input and scale
    output columns afterwards.  Input arrives host-pre-transposed as lnT
    [ch_chunk, 128, (s, n)] (s-permuted so l=0 lands in the last 64 cols =
    its own PSUM bank):

      xg             = gamma[l_s, ch] * lnT          (per-partition scalars)
      P[ci, (s, n)]  = sum_ch Wo[l_s][ch, ci] * xg   (6 matmuls)
      stat rows      = ones-column matmuls over lnT / lnT^2, the s-group sums
                       accumulated on the PE into one PSUM bank -> [1, n]
      y              = P * broadcast(scale rows)  (+ bias / -mu rank-1 on l=0)

    No on-device transposes or diag tiles; broadcasting the scale rows to
    128 partitions is 9 tiny K=1 PE matmuls into PSUM."""
    nc = bass.Bass("TRN2", target_bir_lowering=False, debug=False, num_devices=H)
    lnT_d = nc.dram_tensor("lnT", [2, 128, SN], BF16, kind="ExternalInput")
    # packed output weights: woe[p, c*384 + l*128 + ci] = Wo[l, c*128+p, ci]
    woe_d = nc.dram_tensor("woe", [128, 2 * (LMAX + 1) * CIN], BF16,
                           kind="ExternalInput")
    # gb[:, 2l+c] = gamma[l, c*128+p]; gb[:, 6] = bo
    gb_d = nc.dram_tensor("gb", [128, 7], F32, kind="ExternalInput")
    bcol_d = nc.dram_tensor("bcol", [128, 2], BF16, kind="ExternalInput")
    y_d = nc.dram_tensor("yT", [CIN, SN], F32, kind="ExternalOutput")

    with tile.TileContext(nc) as tc:
        with (
            tc.tile_pool(name="const", bufs=1) as cpool,
            tc.tile_pool(name="work", bufs=1) as workp,
            tc.tile_pool(name="pp", bufs=1, space="PSUM") as ppp,
            tc.tile_pool(name="pa", bufs=1, space="PSUM") as pap,
            tc.tile_pool(name="pst", bufs=1, space="PSUM") as pstp,
        ):
            W3 = (LMAX + 1) * CIN   # 384: per-chunk block in woe
            lnT = [workp.tile([128, SN], BF16, tag=f"lnT{c}", name=f"lnT{c}")
                   for c in range(2)]
            woe_sb = cpool.tile([128, 2 * W3], BF16, tag="woe", name="woe")
            gb_sb = cpool.tile([128, 7], F32, tag="gb", name="gb")
            bcol_sb = cpool.tile([128, 2], BF16, tag="bcol", name="bcol")
            nc.sync.dma_start(lnT[0][:], lnT_d[0, :, :])
            nc.scalar.dma_start(lnT[1][:], lnT_d[1, :, :])
            nc.sync.dma_start(woe_sb[:], woe_d[:])
            # gpsimd queue: gb first (feeds the gamma input-fold), small
            # consts via memset, bcol last (only feeds the beta bias)
            nc.gpsimd.dma_start(gb_sb[:], gb_d[:])
            onesc = cpool.tile([128, 1], BF16, tag="onesc", name="onesc")
            nc.gpsimd.memset(onesc[:], 1.0)
            ones1 = cpool.tile([1, 128], BF16, tag="ones1", name="ones1")
            nc.gpsimd.memset(ones1[:], 1.0)
            epsr = cpool.tile([1, 1], F32, tag="epsr", name="epsr")
            nc.gpsimd.memset(epsr[:], EPS)
            nc.gpsimd.dma_start(bcol_sb[:], bcol_d[:])

            import contextlib as _ctl
            _loop = tc.For_i(0, loop_R, 1) if loop_R else _ctl.nullcontext()
            with _loop:
                # ---- gamma folded into the INPUT columns (per-partition
                # scalars), so projection starts as soon as woe lands
                xg = [workp.tile([128, SN], BF16, tag=f"xg{c}", name=f"xg{c}")
                      for c in range(2)]
                for c in range(2):
                    nc.vector.tensor_scalar_mul(
                        xg[c][:, 0:NL1], lnT[c][:, 0:NL1],
                        gb_sb[:, 2 + c:3 + c])
                    nc.gpsimd.tensor_scalar_mul(
                        xg[c][:, NL1:NL2], lnT[c][:, NL1:NL2],
                        gb_sb[:, 4 + c:5 + c])
                    nc.vector.tensor_scalar_mul(
                        xg[c][:, NL2:SN], lnT[c][:, NL2:SN],
                        gb_sb[:, c:c + 1])
                # negated gamma-l0 column (bf16) for the mean rank-1 weight sum
                ngcol = cpool.tile([128, 2], BF16, tag="ngcol", name="ngcol")
                for c in range(2):
                    nc.gpsimd.tensor_scalar_mul(ngcol[:, c:c + 1],
                                                gb_sb[:, c:c + 1], -1.0)

                # ---- squares for the RMS stats (c0 split Act/DVE, c1 DVE)
                sq = [workp.tile([128, SN], BF16, tag=f"sq{c}", name=f"sq{c}")
                      for c in range(2)]
                nc.scalar.activation(sq[0][:, 0:256], lnT[0][:, 0:256],
                                     AF.Square)
                nc.vector.tensor_tensor(out=sq[0][:, 256:SN],
                                        in0=lnT[0][:, 256:SN],
                                        in1=lnT[0][:, 256:SN], op=ALU.mult)
                nc.vector.tensor_tensor(out=sq[1][:], in0=lnT[1][:],
                                        in1=lnT[1][:], op=ALU.mult)

                # ---- all stat sums accumulate into ONE psum bank (psr):
                #   [0:64) ss1 (l=1 sq-sum), [64:128) ss2, [128:192) sq0-sum,
                #   [192:256) mu-sum, [256:384) -sum_ch g0*Wo0 (rank-1 row)
                # only the very first matmul sets start=True (bank pending-
                # zero); each region's first write is zeroed by that flag
                psr = pstp.tile([1, 6 * NR], F32, tag="psr", name="psr")
                pss1 = psr[:, 0:NR]
                pss2 = psr[:, NR:2 * NR]
                pq1 = psr[:, 2 * NR:3 * NR]
                pmu = psr[:, 3 * NR:4 * NR]
                pnw = psr[:, 4 * NR:6 * NR]
                first = True
                for c in range(2):
                    for j in range(3):
                        nc.tensor.matmul(
                            pss1, lhsT=onesc[:],
                            rhs=sq[c][:, j * NR:(j + 1) * NR],
                            start=first, stop=(c == 1 and j == 2),
                            skip_group_check=True)
                        first = False
                    for j in range(3, 8):
                        nc.tensor.matmul(
                            pss2, lhsT=onesc[:],
                            rhs=sq[c][:, j * NR:(j + 1) * NR],
                            start=False, stop=(c == 1 and j == 7),
                            skip_group_check=True)
                    nc.tensor.matmul(pq1, lhsT=onesc[:], rhs=sq[c][:, NL2:SN],
                                     start=False, stop=(c == 1),
                                     skip_group_check=True)
                    nc.tensor.matmul(pmu, lhsT=onesc[:], rhs=lnT[c][:, NL2:SN],
                                     start=False, stop=(c == 1),
                                     skip_group_check=True)

                # ---- projection of the gamma-folded input (c-accumulated)
                pP0 = ppp.tile([128, NL2], F32, tag="pP0", name="pP0")
                pP1 = ppp.tile([128, NR], F32, tag="pP1", name="pP1")
                for c in range(2):
                    nc.tensor.matmul(
                        pP0[:, 0:NL1],
                        lhsT=woe_sb[:, c * W3 + CIN:c * W3 + 2 * CIN],
                        rhs=xg[c][:, 0:NL1],
                        start=(c == 0), stop=(c == 1), skip_group_check=True)
                    nc.tensor.matmul(
                        pP0[:, NL1:NL2],
                        lhsT=woe_sb[:, c * W3 + 2 * CIN:c * W3 + 3 * CIN],
                        rhs=xg[c][:, NL1:NL2],
                        start=False, stop=(c == 1), skip_group_check=True)
                    nc.tensor.matmul(
                        pP1[:],
                        lhsT=woe_sb[:, c * W3:c * W3 + CIN],
                        rhs=xg[c][:, NL2:SN],
                        start=(c == 0), stop=False, skip_group_check=True)
                # rank-1 weight row: -sum_ch gamma0[ch] Wo0[ch, ci]
                for c in range(2):
                    nc.tensor.matmul(pnw, lhsT=ngcol[:, c:c + 1],
                                     rhs=woe_sb[:, c * W3:c * W3 + CIN],
                                     start=False, stop=(c == 1),
                                     skip_group_check=True)

                # ---- beta @ Wo0 + bo  (per-partition bias column for l=0)
                pbw = pstp.tile([CIN, 1], F32, tag="pbw", name="pbw")
                for c in range(2):
                    nc.tensor.matmul(pbw[:], lhsT=woe_sb[:, c * W3:c * W3 + CIN],
                                     rhs=bcol_sb[:, c:c + 1],
                                     start=(c == 0), stop=(c == 1))
                bo0 = workp.tile([CIN, 1], F32, tag="bo0", name="bo0")
                nc.vector.tensor_tensor(out=bo0[:], in0=pbw[:],
                                        in1=gb_sb[:, 6:7], op=ALU.add)

                # ---- stats rows -> scale rows (Act queue ordered by need)
                sd1 = workp.tile([1, NR], F32, tag="sd1", name="sd1")
                sd2 = workp.tile([1, NR], F32, tag="sd2", name="sd2")
                nc.scalar.activation(sd1[:], pss1, AF.Sqrt,
                                     scale=1.0 / (3 * CH), bias=epsr[0:1, 0:1])
                nc.scalar.activation(sd2[:], pss2, AF.Sqrt,
                                     scale=1.0 / (5 * CH), bias=epsr[0:1, 0:1])
                mu_f = workp.tile([1, NR], F32, tag="mu_f", name="mu_f")
                nc.scalar.activation(mu_f[:], pmu, AF.Copy, scale=1.0 / CH)
                # mean row in bf16 (the later rstd column-scale multiplies the
                # whole P0, including this rank-1, so the row is plain mu)
                mrow = workp.tile([1, NR], BF16, tag="mrow", name="mrow")
                nc.scalar.activation(mrow[:], pmu, AF.Copy, scale=1.0 / CH)
                nws0 = workp.tile([1, CIN], BF16, tag="nws0", name="nws0")
                nc.scalar.copy(nws0[:], pnw)
                rr1 = workp.tile([1, NR], BF16, tag="rr1", name="rr1")
                rr2 = workp.tile([1, NR], BF16, tag="rr2", name="rr2")
                m2 = workp.tile([1, NR], F32, tag="m2", name="m2")
                nc.vector.tensor_tensor(out=m2[:], in0=mu_f[:], in1=mu_f[:],
                                        op=ALU.mult)
                m2e = workp.tile([1, NR], F32, tag="m2e", name="m2e")
                nc.gpsimd.tensor_scalar(out=m2e[:], in0=m2[:], scalar1=1.0,
                                        scalar2=-EPS, op0=ALU.mult, op1=ALU.add)
                ve = workp.tile([1, NR], F32, tag="ve", name="ve")
                nc.vector.scalar_tensor_tensor(
                    out=ve[:], in0=pq1, scalar=1.0 / CH, in1=m2e[:],
                    op0=ALU.mult, op1=ALU.subtract)
                with nc.allow_low_precision(reason="bf16 scales, as baseline"):
                    nc.vector.reciprocal(rr1[:], sd1[:])
                    nc.vector.reciprocal(rr2[:], sd2[:])
                sd0 = workp.tile([1, NR], F32, tag="sd0", name="sd0")
                nc.scalar.activation(sd0[:], ve[:], AF.Sqrt)
                rstd = workp.tile([1, NR], BF16, tag="rstd", name="rstd")
                with nc.allow_low_precision(reason="bf16 scales, as baseline"):
                    nc.vector.reciprocal(rstd[:], sd0[:])

                # ---- broadcast scale rows to 128 partitions (K=1 matmuls)
                pA0 = pap.tile([128, NL2], F32, tag="pA0", name="pA0")
                pA1 = pap.tile([128, NR], F32, tag="pA1", name="pA1")
                for j in range(8):
                    row = rr1 if j < 3 else rr2
                    nc.tensor.matmul(pA0[:, j * NR:(j + 1) * NR],
                                     lhsT=ones1[:], rhs=row[:],
                                     start=True, stop=True,
                                     skip_group_check=True)
                nc.tensor.matmul(pA1[:], lhsT=ones1[:], rhs=rstd[:],
                                 start=True, stop=True)
                # mean rank-1 into the l=0 projection:  += (-sum g0 Wo0) x mu
                nc.tensor.matmul(pP1[:], lhsT=nws0[:], rhs=mrow[:],
                                 start=False, stop=True, skip_group_check=True)

                # ---- extract P to SBUF (Act+DVE split), scale by the A rows
                psb0 = workp.tile([128, NL2], BF16, tag="psb0", name="psb0")
                nc.scalar.copy(psb0[:, 0:256], pP0[:, 0:256])
                nc.vector.tensor_copy(out=psb0[:, 256:NL2], in_=pP0[:, 256:NL2])
                psb1 = workp.tile([128, NR], BF16, tag="psb1", name="psb1")
                nc.scalar.copy(psb1[:], pP1[:])
                y_sb = workp.tile([CIN, SN], F32, tag="ysb", name="ysb")
                y1t = workp.tile([128, NR], F32, tag="y1t", name="y1t")
                nc.vector.tensor_tensor(out=y1t[:], in0=psb1[:], in1=pA1[:],
                                        op=ALU.mult)
                nc.gpsimd.tensor_scalar_add(y_sb[:, NL2:SN], y1t[:],
                                            bo0[:, 0:1])
                nc.scalar.dma_start(y_d[:, NL2:SN], y_sb[:, NL2:SN])
                nc.vector.tensor_tensor(out=y_sb[:, 0:NL2], in0=psb0[:],
                                        in1=pA0[:], op=ALU.mult)
                nc.sync.dma_start(y_d[:, 0:NL2], y_sb[:, 0:NL2])

                if debug_taps:
                    dbg_rows_d = nc.dram_tensor("dbg_rows", [1, 6 * NR], F32,
                                                kind="ExternalOutput")
                    dbg_psb_d = nc.dram_tensor("dbg_psb", [128, NL2 + NR], F32,
                                               kind="ExternalOutput")
                    dbg_pa_d = nc.dram_tensor("dbg_pa", [128, NL2 + NR], F32,
                                              kind="ExternalOutput")
                    drows = workp.tile([1, 6 * NR], F32, tag="drows", name="drows")
                    nc.vector.tensor_copy(out=drows[:, 0:NR], in_=rr1[:])
                    nc.vector.tensor_copy(out=drows[:, NR:2 * NR], in_=rr2[:])
                    nc.vector.tensor_copy(out=drows[:, 2 * NR:3 * NR], in_=rstd[:])
                    nc.vector.tensor_copy(out=drows[:, 3 * NR:4 * NR], in_=mu_f[:])
                    nc.vector.tensor_copy(out=drows[:, 4 * NR:5 * NR], in_=mrow[:])
                    nc.vector.tensor_copy(out=drows[:, 5 * NR:6 * NR], in_=ve[:])
                    nc.sync.dma_start(dbg_rows_d[:], drows[:])
                    dpsb = workp.tile([128, NL2 + NR], F32, tag="dpsb", name="dpsb")
                    nc.vector.tensor_copy(out=dpsb[:, 0:NL2], in_=psb0[:])
                    nc.vector.tensor_copy(out=dpsb[:, NL2:], in_=psb1[:])
                    nc.sync.dma_start(dbg_psb_d[:], dpsb[:])
                    dpa = workp.tile([128, NL2 + NR], F32, tag="dpa", name="dpa")
                    nc.vector.tensor_copy(out=dpa[:, 0:NL2], in_=pA0[:])
                    nc.vector.tensor_copy(out=dpa[:, NL2:], in_=pA1[:])
                    nc.sync.dma_start(dbg_pa_d[:], dpa[:])

    _split_multiwaits(nc)
    return nc


# ------------------------------------------------------------------ host side
def _prep_inputs(inputs: dict[str, np.ndarray]):
    """Split the full inputs into per-core in_maps for the two phases
    (index bookkeeping and value re-layout only; all arithmetic on device)."""
    q = np.asarray(inputs["q"], np.float32).reshape(N, S, CIN)
    k = np.asarray(inputs["k"], np.float32).reshape(N, S, CIN)
    v = np.asarray(inputs["v"], np.float32).reshape(N, S, CIN)
    # host pre-transpose to [i, (s, m)] and cast to bf16; vT mt-major
    qT = np.ascontiguousarray(q.transpose(2, 1, 0).reshape(CIN, S * N)).astype(NP_BF16)
    kT = np.ascontiguousarray(k.transpose(2, 1, 0).reshape(CIN, S * N)).astype(NP_BF16)
    vT = np.ascontiguousarray(
        v.reshape(NT, 128, S, CIN).transpose(3, 0, 2, 1).reshape(CIN, S * N)
    ).astype(NP_BF16)
    env = np.asarray(inputs["envelope"], np.float32)
    attn_bias = np.asarray(inputs["attn_bias"], np.float32)
    a_idx = np.asarray(inputs["atom_index"]).astype(np.int64)
    b_idx = np.asarray(inputs["batch_index"]).astype(np.int64)
    e_map = np.asarray(inputs["edge_map_tab"]).astype(np.int64)
    Wq = np.asarray(inputs["Wq"], np.float32)
    Wk = np.asarray(inputs["Wk"], np.float32)
    Wv = np.asarray(inputs["Wv"], np.float32)
    bq = np.asarray(inputs["bq"], np.float32)
    bk = np.asarray(inputs["bk"], np.float32)
    bv = np.asarray(inputs["bv"], np.float32)
    gamma = np.asarray(inputs["gamma"], np.float32)
    beta = np.asarray(inputs["beta"], np.float32)
    Wo = np.asarray(inputs["Wo"], np.float32)
    bo = np.asarray(inputs["bo"], np.float32)

    # ---- slot layout for the (atom, segment) cells
    cell = a_idx * NSEG + b_idx                      # [E]
    order = np.argsort(cell, kind="stable")
    cell_s = cell[order]
    counts = np.bincount(cell_s, minlength=N * NSEG)
    L2 = int(counts.max())
    starts = np.zeros(N * NSEG, np.int64)
    starts[1:] = np.cumsum(counts)[:-1]
    rank = np.arange(E) - starts[cell_s]             # rank within cell
    m_s = cell_s // NSEG
    g_s = cell_s % NSEG
    p_s = m_s % 128
    t_s = m_s // 128
    col = (t_s * NSEG + g_s) * L2 + rank             # free-dim position
    Wd = NT * NSEG * L2
    env_e = env[e_map]                               # value gather (re-layout)
    envS = np.zeros((128, Wd), NP_BF16)
    envS[p_s, col] = env_e[order]
    bS_all = []
    for h in range(H):
        bs = np.zeros((128, Wd), NP_BF16)
        bs[p_s, col] = attn_bias[h, e_map][order]
        bS_all.append(bs)

    # ---- per-head weight slices, expanded per spherical component, [i,(s,o)]
    WqE = Wq[L_OF_M]                                 # [9, CIN, CH]
    WkE = Wk[L_OF_M]
    WvE = Wv[L_OF_M]

    in_maps1 = []
    for h in range(H):
        sl = slice(h * D, (h + 1) * D)
        in_maps1.append({
            "qT": qT, "kT": kT, "vT": vT,
            "wq": np.ascontiguousarray(
                WqE[:, :, sl].transpose(1, 0, 2).reshape(CIN, S * D)).astype(NP_BF16),
            "wk": np.ascontiguousarray(
                WkE[:, :, sl].transpose(1, 0, 2).reshape(CIN, S * D)).astype(NP_BF16),
            "wv": np.ascontiguousarray(
                WvE[:, :, sl].transpose(1, 0, 2).reshape(CIN, S * D)).astype(NP_BF16),
            "bqkv": np.ascontiguousarray(
                np.stack([bq[sl], bk[sl], bv[sl]], axis=1)
            ),
            "bvrow": np.ascontiguousarray(bv[sl].reshape(1, D)),
            "envs": envS,
            "bs": bS_all[h],
        })

    # ---- phase-2 constants
    # gb[p, 2l+c] = gamma[l, c*128+p];  gb[p, 6] = bo[p];  bcol[p, c] = beta
    gb = np.zeros((128, 7), np.float32)
    for l in range(LMAX + 1):
        for c in range(2):
            gb[:, 2 * l + c] = gamma[l, c * 128:(c + 1) * 128]
    gb[:, 6] = bo
    bcol = np.stack([beta[0:128], beta[128:256]], axis=1).astype(NP_BF16)
    # woe[p, c*384 + l*128 + ci] = Wo[l, c*128+p, ci]
    woe = np.zeros((128, 2 * (LMAX + 1) * CIN), NP_BF16)
    W3 = (LMAX + 1) * CIN
    for c in range(2):
        woe[:, c * W3:(c + 1) * W3] = Wo[:, c * 128:(c + 1) * 128, :].transpose(
            1, 0, 2).reshape(128, W3).astype(NP_BF16)
    p2_const = {"gb": gb, "bcol": bcol, "woe": woe}
    return in_maps1, L2, p2_const


def _reorder_ao(ao_all: list[np.ndarray]) -> list[np.ndarray]:
    """[h][N, (s,d)] -> per-core lnT [2, 128, (s_perm, n)] chunks, s-permuted
    channel-major slices for phase 2 (pure data movement)."""
    full = np.stack([np.asarray(a).reshape(N, S, D) for a in ao_all], axis=2)
    full = full.reshape(N, S, CH)                                 # [N, S, CH]
    out = []
    for c in range(H):
        xs = full[c * NR:(c + 1) * NR][:, SPERM, :]               # [64, 9, CH]
        t = xs.transpose(2, 1, 0).reshape(2, 128, SN)             # [CH, 9, 64]
        out.append(np.ascontiguousarray(t).astype(NP_BF16))
    return out


_BUILD_CACHE: dict = {}


def kernel(**inputs) -> np.ndarray:
    in_maps1, L2, p2_const = _prep_inputs(inputs)
    nc1 = _BUILD_CACHE.get(("p1", L2))
    if nc1 is None:
        nc1 = build_bass(L2)
        _BUILD_CACHE[("p1", L2)] = nc1
    res1 = run_bass_kernel_spmd(nc1, in_maps1, core_ids=list(range(H)))
    lnin_slices = _reorder_ao([r["ao"] for r in res1.results])

    nc2 = _BUILD_CACHE.get("p2")
    if nc2 is None:
        nc2 = build_phase2()
        _BUILD_CACHE["p2"] = nc2
    in_maps2 = [{"lnT": lnin_slices[c], **p2_const} for c in range(H)]
    res2 = run_bass_kernel_spmd(nc2, in_maps2, core_ids=list(range(H)))
    # yT [ci, (s_perm, n_local)] per core -> y [N, S, CIN]
    y = np.zeros((N, S, CIN), np.float32)
    for c in range(H):
        yt = res2.results[c]["yT"].reshape(CIN, S, NR)
        y[c * NR:(c + 1) * NR][:, SPERM, :] = yt.transpose(2, 1, 0)
    return np.ascontiguousarray(y)



# revision 24
# speedup vs baseline: 1.0806x; 1.0806x over previous
"""Equivariant attention (gnn_message_passing) on 8 Trainium2 NeuronCores.

Strategy (head-sharded tensor parallel, core c owns head c):

The reference materializes [H, N, E] scores/attn over E=8192 edges. Here the
edge dimension is collapsed onto the N=512 atoms at projection level:

  scores[h, n, e]   = sf[h, n, a_e] + bias[h, edge_map[e]]     (a_e = atom_index)
  attn-softmax per (batch-segment, n) then  out = attn @ vh_edges

factors exactly into dense [N, N] algebra with two tiny per-(segment, atom)
tables (NSEG=16 x N=512):

  D[g, m] = sum_{e in seg g, a_e = m} env_e   * exp(b_e)
  C[g, m] = sum_{e in seg g, a_e = m} env_e^2 * exp(b_e)
  den[g, n]  = sum_m exp(sf[m, n]) * D[g, m]           (one matmul)
  Aagg[m, n] = exp(sf[m, n]) * sum_g C[g, m] / den[g, n]
  out[n, f]  = Aagg^T @ vh[m, f]                        (one matmul)

The running-max subtraction in the reference softmax cancels exactly (up to a
+1e-16 epsilon whose relative effect is ~1e-16) and |scale*sf + b| < 20, so
unnormalized exp is safe in f32.

D/C are built on-device from "slot tensors": host packs per-(atom, segment)
edge lists into a fixed-width [128, 4*16*L2] layout (env & bias values; pads
have env=0 so they vanish), and a single free-axis reduce per table produces
it. Only integer index bookkeeping and value re-layout happen on host.

q/k/v arrive HOST-PRE-TRANSPOSED as qT/kT/vT [CIN, S*N] (channel-major), so
the kernel needs no on-device input transposes: projections read qT slices
directly.  DMAs are issued in first-use order so the PE starts ~1.5us in.

Phase 2 (per-core 64-atom slice): LN + output projection in yT [ci, (s, n)]
orientation.  The per-atom LN scaling is fused into the PE transpose by
replacing the identity operand with diag(scale) (the mean subtraction rides
along as a rank-1 ones x (-mu*rstd) matmul); gamma is folded into the output
weights on device, beta/bo become per-partition biases of the PSUM extract.
Host un-transposes the yT output for free.

Both phases are bf16 on the big operands (inputs, weights, intermediates
bound for matmuls); f32 is kept for the softmax denominators, LN statistics
and the final output.  HW-verified rel err ~8e-3 (gate 2e-2).
"""

import os
import numpy as np

import concourse.bass as bass
import concourse.tile as tile
from concourse import mybir
from concourse.bass_utils import run_bass_kernel_spmd
from concourse.masks import make_identity

# ---------------------------------------------------------------- constants
H, LMAX, NSEG = 8, 2, 16
S = (LMAX + 1) ** 2          # 9 spherical components
N, E, CIN, CH = 512, 8192, 128, 256
D = CH // H                  # 32 per-head channels
F = S * D                    # 288 per-head feature width
NT = N // 128                # 4 atom tiles
NR = N // H                  # 64 atoms per core in the LN/out stage
EPS = 1e-7
SCALE = float(np.sqrt(D / 3.0) / D)
L_OF_M = np.floor(np.sqrt(np.arange(S))).astype(np.int64)
F32 = mybir.dt.float32
F32R = mybir.dt.float32r
BF16 = mybir.dt.bfloat16
AF = mybir.ActivationFunctionType
ALU = mybir.AluOpType

import ml_dtypes
NP_BF16 = ml_dtypes.bfloat16

_DBG = bool(int(os.environ.get("KBDBG", "0")))


def _split_multiwaits(nc: bass.Bass, limit: int = 1):
    """This walrus build rejects instructions carrying more than one semaphore
    wait (and Drains carrying any). Hoist excess waits onto NOPs inserted just
    before the instruction on the same engine - semantically identical."""
    for f in nc.m.functions:
        for blk in f.blocks:
            changed = False
            out = []
            for inst in blk.instructions:
                si = inst.sync_info
                waits = list(si.on_wait) if si is not None else []
                keep = 0 if inst.opcode == "Drain" else limit
                if len(waits) > keep:
                    hoist = waits[: len(waits) - keep]
                    rest = waits[len(waits) - keep:]
                    for w in hoist:
                        nop = mybir.InstNoOp(
                            name=f"{inst.name}-w{len(out)}", ins=[], outs=[]
                        )
                        nop.engine = inst.engine
                        nop.sync_info = mybir.SyncInfo(on_wait=[w], on_update=[])
                        out.append(nop)
                    inst.sync_info = mybir.SyncInfo(
                        on_wait=rest, on_update=list(si.on_update)
                    )
                    changed = True
                out.append(inst)
            if changed:
                blk.instructions = out


def build_bass(L2: int, loop_R: int | None = None) -> bass.Bass:
    """One SPMD program; per-core data (weight slices, bias slots) comes in as
    inputs. L2 = slot width per (atom, segment) cell."""
    W = NT * NSEG * L2  # slot tensor free width per partition

    nc = bass.Bass("TRN2", target_bir_lowering=False, debug=False, num_devices=H)

    # ------------------------------------------------------------- tensors
    # host-pre-transposed bf16 inputs: qT/kT [i, (s, m)]; vT mt-major
    # [i, (t, s, j)] so each quarter is a contiguous DMA
    qT_d = nc.dram_tensor("qT", [CIN, S * N], BF16, kind="ExternalInput")
    kT_d = nc.dram_tensor("kT", [CIN, S * N], BF16, kind="ExternalInput")
    vT_d = nc.dram_tensor("vT", [CIN, S * N], BF16, kind="ExternalInput")
    wq_d = nc.dram_tensor("wq", [CIN, S * D], BF16, kind="ExternalInput")  # [i,(s,o)]
    wk_d = nc.dram_tensor("wk", [CIN, S * D], BF16, kind="ExternalInput")
    wv_d = nc.dram_tensor("wv", [CIN, S * D], BF16, kind="ExternalInput")
    bqkv_d = nc.dram_tensor("bqkv", [D, 3], F32, kind="ExternalInput")
    bvrow_d = nc.dram_tensor("bvrow", [1, D], F32, kind="ExternalInput")
    envs_d = nc.dram_tensor("envs", [128, W], BF16, kind="ExternalInput")
    bs_d = nc.dram_tensor("bs", [128, W], BF16, kind="ExternalInput")
    ao_d = nc.dram_tensor("ao", [N, F], BF16, kind="ExternalOutput")

    with tile.TileContext(nc) as tc:
        with (
            tc.tile_pool(name="const", bufs=1) as cpool,
            tc.tile_pool(name="feat", bufs=1) as featp,
            tc.tile_pool(name="work", bufs=1) as workp,
            tc.tile_pool(name="aop", bufs=1) as aop,
            tc.tile_pool(name="ps", bufs=4, space="PSUM") as psp,
            tc.tile_pool(name="pop", bufs=1, space="PSUM") as pop,
        ):
            def ps_tile(shape):
                return psp.tile(shape, F32, tag="ps", name="ps")

            # -------- DMA issue, first-use order, spread over engines ------
            # sync queue:   wq, qT thirds, vT quarters (mt-major layout)
            # scalar queue: wk, wv, kT thirds, envs, bs
            # gpsimd queue: tiny constants
            TH = 3 * N
            wq_sb = cpool.tile([CIN, S * D], BF16, tag="wq", name="wq")
            wk_sb = cpool.tile([CIN, S * D], BF16, tag="wk", name="wk")
            wv_sb = cpool.tile([CIN, S * D], BF16, tag="wv", name="wv")
            qTc = [cpool.tile([CIN, TH], BF16, tag=f"qT{t}", name=f"qT{t}")
                   for t in range(3)]
            kTc = [cpool.tile([CIN, TH], BF16, tag=f"kT{t}", name=f"kT{t}")
                   for t in range(3)]
            # vT quarter mt holds [i, (s, m_local)] for atom tile mt
            vTq = [cpool.tile([CIN, S * 128], BF16, tag=f"vT{m}", name=f"vT{m}")
                   for m in range(NT)]
            envs_sb = cpool.tile([128, W], BF16, tag="envs", name="envs")
            bs_sb = cpool.tile([128, W], BF16, tag="bs", name="bs")

            def vtq_src(m):
                return vT_d[:].rearrange("i (t x) -> i t x", t=NT)[:, m, :]

            nc.sync.dma_start(wq_sb[:], wq_d[:])
            for t in range(3):
                nc.sync.dma_start(qTc[t][:], qT_d[:, t * TH:(t + 1) * TH])
            nc.sync.dma_start(vTq[0][:], vtq_src(0))
            nc.sync.dma_start(vTq[1][:], vtq_src(1))
            nc.scalar.dma_start(wk_sb[:], wk_d[:])
            for t in range(3):
                nc.scalar.dma_start(kTc[t][:], kT_d[:, t * TH:(t + 1) * TH])
            nc.scalar.dma_start(wv_sb[:], wv_d[:])
            nc.gpsimd.dma_start(envs_sb[:], envs_d[:])
            nc.gpsimd.dma_start(bs_sb[:], bs_d[:])
            nc.scalar.dma_start(vTq[2][:], vtq_src(2))
            nc.scalar.dma_start(vTq[3][:], vtq_src(3))
            bqkv_sb = cpool.tile([D, 3], F32, tag="bqkv", name="bqkv")
            nc.gpsimd.dma_start(bqkv_sb[:], bqkv_d[:])
            bvrow_sb = cpool.tile([128, D], F32, tag="bvrow", name="bvrow")
            nc.gpsimd.dma_start(bvrow_sb[:], bvrow_d[0:1, :].to_broadcast([128, D]))
            ident = cpool.tile([128, 128], F32, tag="ident", name="ident")
            make_identity(nc, ident[:])
            eps16 = cpool.tile([1, NSEG], F32, tag="eps16", name="eps16")
            nc.gpsimd.memset(eps16[:], 1e-16)
            ones_n = cpool.tile([1, N], F32, tag="ones_n", name="ones_n")
            nc.gpsimd.memset(ones_n[:], 1.0)

            import contextlib as _ctl
            _loop = tc.For_i(0, loop_R, 1) if loop_R else _ctl.nullcontext()
            with _loop:
                copy_engines = [nc.scalar, nc.vector]
                cp_i = 0

                def copy_alt(dst_ap, src_ap):
                    nonlocal cp_i
                    eng = copy_engines[cp_i % 2]
                    cp_i += 1
                    if eng is nc.scalar:
                        eng.copy(dst_ap, src_ap)
                    else:
                        eng.tensor_copy(out=dst_ap, in_=src_ap)

                def copy_dve(dst_ap, src_ap, eng=None):
                    if eng is nc.scalar:
                        eng.copy(dst_ap, src_ap)
                    else:
                        nc.vector.tensor_copy(out=dst_ap, in_=src_ap)

                # ------------------------------ D / C tables (early, off-path)
                ebs = workp.tile([128, W], F32, tag="ebs", name="ebs")
                nc.scalar.activation(ebs[:], bs_sb[:], AF.Exp)
                wD = workp.tile([128, W], F32, tag="wD", name="wD")
                nc.vector.tensor_tensor(out=wD[:], in0=envs_sb[:], in1=ebs[:], op=ALU.mult)
                wC = workp.tile([128, W], F32, tag="wC", name="wC")
                nc.vector.tensor_tensor(out=wC[:], in0=wD[:], in1=envs_sb[:], op=ALU.mult)
                d_t = featp.tile([128, NT * NSEG], F32, tag="d_t", name="d_t")  # [m_p, (mt, g)]
                c_t = featp.tile([128, NT * NSEG], F32, tag="c_t", name="c_t")
                with nc.allow_low_precision(reason="f32r is 32-bit storage"):
                    nc.vector.reduce_sum(
                        out=d_t[:].rearrange("p (t g) -> p t g", t=NT).bitcast(F32R),
                        in_=wD[:].rearrange("p (t g j) -> p t g j", t=NT, g=NSEG),
                        axis=mybir.AxisListType.X,
                    )
                nc.vector.reduce_sum(
                    out=c_t[:].rearrange("p (t g) -> p t g", t=NT),
                    in_=wC[:].rearrange("p (t g j) -> p t g j", t=NT, g=NSEG),
                    axis=mybir.AxisListType.X,
                )
                # ------- fq / fk chunk-streamed projection + psf accumulation
                # chunk layout: rows (s_local*32+o), chunks s=0..2 / 3..5 / 6..8
                # (96 rows per chunk so matmul outs land at base 0/32/64);
                # psf[mt] accumulates across chunks in 4 held banks (tags
                # shared with the po accumulators, which start strictly later)
                fq = [featp.tile([96, N], BF16, tag=f"fq{c}", name=f"fq{c}")
                      for c in range(3)]
                fk = [featp.tile([96, N], BF16, tag=f"fk{c}", name=f"fk{c}")
                      for c in range(3)]
                psf = [pop.tile([128, N], F32, tag=f"acc{mt}", name=f"psf{mt}")
                       for mt in range(NT)]
                # within chunk 0 the s components sit in row order (1, 2, 0)
                # so the biased s=0 rows are 64-aligned for the PSUM read
                # (fk uses the same permutation, so scores are unchanged)
                ROWOF = {0: 2, 1: 0, 2: 1}
                for chunk in range(3):
                    for t_c, w_sb, f_dst, t_idx in ((qTc, wq_sb, fq, 0),
                                                    (kTc, wk_sb, fk, 1)):
                        pp = ps_tile([96, N])
                        for j in range(3):
                            s = chunk * 3 + j
                            r = ROWOF[j] if chunk == 0 else j
                            nc.tensor.matmul(
                                pp[r * D:(r + 1) * D, :],
                                lhsT=w_sb[:, s * D:(s + 1) * D],
                                rhs=t_c[chunk][:, j * N:(j + 1) * N],
                                start=True, stop=True,
                            )
                        ceng = nc.scalar if t_idx == 0 else nc.vector
                        if chunk == 0:
                            # bias on s=0 rows (l=0 invariant component)
                            copy_dve(f_dst[0][0:64, :], pp[0:64, :], ceng)
                            nc.vector.tensor_scalar_add(
                                f_dst[0][64:96, :], pp[64:96, :],
                                bqkv_sb[:, t_idx:t_idx + 1])
                        else:
                            copy_dve(f_dst[chunk][:], pp[:], ceng)
                    for mt in range(NT):
                        nc.tensor.matmul(
                            psf[mt][:],
                            lhsT=fk[chunk][:, mt * 128:(mt + 1) * 128],
                            rhs=fq[chunk][:],
                            start=(chunk == 0), stop=(chunk == 2),
                            skip_group_check=True,
                        )

                # C transposed to [g, m]
                c_sb = featp.tile([NSEG, N], F32, tag="c_sb", name="c_sb")
                for mt in range(NT):
                    pc = ps_tile([NSEG, 128])
                    nc.tensor.transpose(
                        pc[:], c_t[:, mt * NSEG:(mt + 1) * NSEG], ident[:]
                    )
                    nc.vector.tensor_copy(out=c_sb[:, mt * 128:(mt + 1) * 128].bitcast(F32R), in_=pc[:])

                # -------------------------------- vhn [m, (s,o)] per m-tile
                vhn = [featp.tile([128, F], BF16, tag=f"vhn{mt}", name=f"vhn{mt}") for mt in range(NT)]
                for mt in range(NT):
                    pv = ps_tile([128, F])
                    for s in range(S):
                        nc.tensor.matmul(
                            pv[:, s * D:(s + 1) * D],
                            lhsT=vTq[mt][:, s * 128:(s + 1) * 128],
                            rhs=wv_sb[:, s * D:(s + 1) * D],
                            start=True, stop=True,
                        )
                    nc.vector.tensor_copy(out=vhn[mt][:, D:F], in_=pv[:, D:F])
                    nc.vector.tensor_tensor(
                        out=vhn[mt][:, 0:D],
                        in0=pv[:, 0:D], in1=bvrow_sb[:], op=ALU.add,
                    )

                # ------- exp + denominator, split in n-halves (h-major) so
                # the dd -> aggt -> att-out chain pipelines per half
                HN = N // 2
                exp_sf = [featp.tile([128, N], F32, tag=f"esf{mt}",
                                     name=f"esf{mt}") for mt in range(NT)]
                pden = [ps_tile([NSEG, HN]) for h in range(2)]
                for h in range(2):
                    nc.tensor.matmul(
                        pden[h][:], lhsT=eps16[:].bitcast(F32R),
                        rhs=ones_n[:, 0:HN].bitcast(F32R), start=True, stop=False,
                        skip_group_check=True,
                    )
                for mt in range(NT):
                    nc.scalar.activation(exp_sf[mt][:].bitcast(F32R), psf[mt][:],
                                         AF.Exp, scale=SCALE)
                    for h in range(2):
                        nc.tensor.matmul(
                            pden[h][:],
                            lhsT=d_t[:, mt * NSEG:(mt + 1) * NSEG].bitcast(F32R),
                            rhs=exp_sf[mt][:, h * HN:(h + 1) * HN].bitcast(F32R),
                            start=False, stop=(mt == NT - 1),
                            skip_group_check=True,
                        )

                # ------- per half: dd; per m-tile: aggt mult; att-out
                aggt = [[featp.tile([128, HN], BF16, tag=f"agg{mt}h{h}",
                                    name=f"agg{mt}h{h}") for h in range(2)]
                        for mt in range(NT)]
                po = [pop.tile([128, F], F32, tag=f"acc{nt}", name=f"po{nt}")
                      for nt in range(NT)]
                dd = [featp.tile([NSEG, HN], F32, tag=f"ddh{h}", name=f"ddh{h}")
                      for h in range(2)]
                for h in range(2):
                    with nc.allow_low_precision(reason="f32r is 32-bit storage"):
                        nc.vector.reciprocal(dd[h][:].bitcast(F32R), pden[h][:])
                    for mt in range(NT):
                        pT = ps_tile([128, HN])
                        nc.tensor.matmul(
                            pT[:],
                            lhsT=c_sb[:, mt * 128:(mt + 1) * 128].bitcast(F32R),
                            rhs=dd[h][:].bitcast(F32R),
                            start=True, stop=True,
                        )
                        nc.vector.tensor_tensor(
                            out=aggt[mt][h][:],
                            in0=exp_sf[mt][:, h * HN:(h + 1) * HN],
                            in1=pT[:], op=ALU.mult)
                        for j in range(2):
                            nt = 2 * h + j
                            nc.tensor.matmul(
                                po[nt][:],
                                lhsT=aggt[mt][h][:, j * 128:(j + 1) * 128],
                                rhs=vhn[mt][:],
                                start=(mt == 0), stop=(mt == NT - 1),
                                skip_group_check=True,
                            )
                # two ao DMAs; slice the t axis AFTER the rearrange (the
                # slice-then-rearrange form writes wrong DRAM locations)
                for pair in range(2):
                    ao = aop.tile([128, 2 * F], BF16, tag=f"ao{pair}",
                                  name=f"ao{pair}")
                    for j in range(2):
                        copy_alt(ao[:, j * F:(j + 1) * F], po[2 * pair + j][:])
                    nc.sync.dma_start(
                        ao_d[:].rearrange("(t p) f -> p t f", t=NT)
                        [:, 2 * pair:2 * pair + 2, :],
                        ao[:].rearrange("p (t f) -> p t f", t=2))

    _split_multiwaits(nc)
    return nc


SN = S * NR          # 576 free columns in phase 2, s-permuted (l=1, l=2, l=0)
SPERM = [1, 2, 3, 4, 5, 6, 7, 8, 0]   # column group j -> spherical component s
NL1 = 3 * NR         # 192: end of the l=1 block
NL2 = 8 * NR         # 512: end of the l=2 block (bank boundary)


def build_phase2(loop_R: int | None = None, debug_taps: bool = False) -> bass.Bass:
    """Equivariant layernorm + output projection on a 64-atom slice.

    The LN scaling is a per-(atom, l) scalar, so it commutes with the output
    projection: project the RAW (gamma-folded) input and scale output columns
    afterwards.  Input arrives host-pre-transposed as lnT [ch_chunk, 128,
    (s, n)] (s-permuted so l=0 lands in the last 64 cols = its own bank):

      xg             = gamma[l_s, ch] * lnT          (per-partition scalars)
      P[ci, (s, n)]  = sum_ch Wo[l_s][ch, ci] * xg   (6 matmuls)
      stat rows      = ones-column matmuls over lnT / lnT^2, s-group sums
                       accumulated on the PE -> [1, n] rows
      y              = P * broadcast(scale rows)  (+ bias / -mu rank-1 on l=0)

    Scheduling notes: tile deps are tracker-granular per TILE, so every
    independently produced region gets its own tile; out-DMAs ride the sync
    queue (Act queue stays free); scale rows use Abs_reciprocal_sqrt (one Act
    op); y leaves in bf16 and the host upcasts."""
    nc = bass.Bass("TRN2", target_bir_lowering=False, debug=False, num_devices=H)
    lnT_d = nc.dram_tensor("lnT", [2, 128, SN], BF16, kind="ExternalInput")
    # packed output weights: woe[p, c*384 + l*128 + ci] = Wo[l, c*128+p, ci]
    woe_d = nc.dram_tensor("woe", [128, 2 * (LMAX + 1) * CIN], BF16,
                           kind="ExternalInput")
    # gb[:, 2l+c] = gamma[l, c*128+p]; gb[:, 6] = bo
    gb_d = nc.dram_tensor("gb", [128, 7], F32, kind="ExternalInput")
    bcol_d = nc.dram_tensor("bcol", [128, 2], BF16, kind="ExternalInput")
    y0_d = nc.dram_tensor("y0T", [CIN, NL2], BF16, kind="ExternalOutput")
    y1_d = nc.dram_tensor("y1T", [CIN, NR], BF16, kind="ExternalOutput")

    with tile.TileContext(nc) as tc:
        with (
            tc.tile_pool(name="const", bufs=1) as cpool,
            tc.tile_pool(name="work", bufs=1) as workp,
            tc.tile_pool(name="pp", bufs=1, space="PSUM") as ppp,
            tc.tile_pool(name="pa", bufs=1, space="PSUM") as pap,
            tc.tile_pool(name="pst", bufs=1, space="PSUM") as pstp,
        ):
            W3 = (LMAX + 1) * CIN   # 384: per-chunk block in woe
            lnTa = workp.tile([128, SN], BF16, tag="lnTa", name="lnTa")
            lnTb = workp.tile([128, SN], BF16, tag="lnTb", name="lnTb")
            woe_sb = cpool.tile([128, 2 * W3], BF16, tag="woe", name="woe")
            gb_sb = cpool.tile([128, 7], F32, tag="gb", name="gb")
            bcol_sb = cpool.tile([128, 2], BF16, tag="bcol", name="bcol")
            # all input DMAs on the sync queue (Act queue stays DMA-free)
            nc.sync.dma_start(lnTa[:], lnT_d[0, :, :])
            nc.sync.dma_start(lnTb[:], lnT_d[1, :, :])
            nc.sync.dma_start(woe_sb[:], woe_d[:])
            # gpsimd queue: consts + small DMAs
            onesc = cpool.tile([128, 1], BF16, tag="onesc", name="onesc")
            nc.gpsimd.memset(onesc[:], 1.0)
            ones1 = cpool.tile([1, 128], BF16, tag="ones1", name="ones1")
            nc.gpsimd.memset(ones1[:], 1.0)
            epsr = cpool.tile([1, 1], F32, tag="epsr", name="epsr")
            nc.gpsimd.memset(epsr[:], EPS)
            nc.gpsimd.dma_start(gb_sb[:], gb_d[:])
            nc.gpsimd.dma_start(bcol_sb[:], bcol_d[:])

            import contextlib as _ctl
            _loop = tc.For_i(0, loop_R, 1) if loop_R else _ctl.nullcontext()
            with _loop:
                lnT = [lnTa, lnTb]
                # ---- squares (one tile per writer: no false deps)
                sq0a = workp.tile([128, 256], BF16, tag="sq0a", name="sq0a")
                sq0b = workp.tile([128, SN - 256], BF16, tag="sq0b", name="sq0b")
                sq1 = workp.tile([128, SN], BF16, tag="sq1", name="sq1")
                nc.scalar.activation(sq0a[:], lnTa[:, 0:256], AF.Square)
                nc.vector.tensor_tensor(out=sq0b[:], in0=lnTa[:, 256:SN],
                                        in1=lnTa[:, 256:SN], op=ALU.mult)
                def sq0(lo, hi):
                    if hi <= 256:
                        return sq0a[:, lo:hi]
                    assert lo >= 256
                    return sq0b[:, lo - 256:hi - 256]
                nc.vector.tensor_tensor(out=sq1[:], in0=lnTb[:],
                                        in1=lnTb[:], op=ALU.mult)

                # ---- gamma folded into the INPUT columns (per-partition
                # scalars); l1/l2 on DVE (fast bf16), l0 on Pool
                xg = [[None] * 3, [None] * 3]   # [c][l]
                for c in range(2):
                    t2 = workp.tile([128, NL2 - NL1], BF16, tag=f"xg{c}l2",
                                    name=f"xg{c}l2")
                    nc.vector.tensor_scalar_mul(t2[:], lnT[c][:, NL1:NL2],
                                                gb_sb[:, 4 + c:5 + c])
                    t1 = workp.tile([128, NL1], BF16, tag=f"xg{c}l1",
                                    name=f"xg{c}l1")
                    nc.vector.tensor_scalar_mul(t1[:], lnT[c][:, 0:NL1],
                                                gb_sb[:, 2 + c:3 + c])
                    t0 = workp.tile([128, NR], BF16, tag=f"xg{c}l0",
                                    name=f"xg{c}l0")
                    nc.gpsimd.tensor_scalar_mul(t0[:], lnT[c][:, NL2:SN],
                                                gb_sb[:, c:c + 1])
                    xg[c] = [t0, t1, t2]
                # negated gamma-l0 column (bf16) for the mean rank-1 weight sum
                ngcol = cpool.tile([128, 2], BF16, tag="ngcol", name="ngcol")
                for c in range(2):
                    nc.gpsimd.tensor_scalar_mul(ngcol[:, c:c + 1],
                                                gb_sb[:, c:c + 1], -1.0)

                # ---- stat sums: ss12 bank [ss1 | ss2], pl0 bank [pq1 | pmu],
                # pnw bank; first matmul into each bank carries start=True,
                # each region's first write is zeroed by the bank pending flag
                ss12 = pstp.tile([1, 2 * NR], F32, tag="ss12", name="ss12")
                pl0 = pstp.tile([1, 2 * NR], F32, tag="pl0", name="pl0")
                pss1, pss2 = ss12[:, 0:NR], ss12[:, NR:2 * NR]
                pq1, pmu = pl0[:, 0:NR], pl0[:, NR:2 * NR]
                pnw = pstp.tile([1, CIN], F32, tag="pnw", name="pnw")
                for c in range(2):
                    sqc = (lambda lo, hi: sq0(lo, hi)) if c == 0 else \
                        (lambda lo, hi: sq1[:, lo:hi])
                    for j in range(3):
                        nc.tensor.matmul(
                            pss1, lhsT=onesc[:], rhs=sqc(j * NR, (j + 1) * NR),
                            start=(c == 0 and j == 0), stop=(c == 1 and j == 2),
                            skip_group_check=True)
                    for j in range(3, 8):
                        nc.tensor.matmul(
                            pss2, lhsT=onesc[:], rhs=sqc(j * NR, (j + 1) * NR),
                            start=False, stop=(c == 1 and j == 7),
                            skip_group_check=True)
                    nc.tensor.matmul(pq1, lhsT=onesc[:], rhs=sqc(NL2, SN),
                                     start=(c == 0), stop=(c == 1),
                                     skip_group_check=True)
                    nc.tensor.matmul(pmu, lhsT=onesc[:], rhs=lnT[c][:, NL2:SN],
                                     start=False, stop=(c == 1),
                                     skip_group_check=True)

                # ---- projection of the gamma-folded input (c-accumulated)
                pP0 = ppp.tile([128, NL2], F32, tag="pP0", name="pP0")
                pP1 = ppp.tile([128, NR], F32, tag="pP1", name="pP1")
                for c in range(2):
                    nc.tensor.matmul(
                        pP0[:, 0:NL1],
                        lhsT=woe_sb[:, c * W3 + CIN:c * W3 + 2 * CIN],
                        rhs=xg[c][1][:],
                        start=(c == 0), stop=(c == 1), skip_group_check=True)
                    nc.tensor.matmul(
                        pP0[:, NL1:NL2],
                        lhsT=woe_sb[:, c * W3 + 2 * CIN:c * W3 + 3 * CIN],
                        rhs=xg[c][2][:],
                        start=False, stop=(c == 1), skip_group_check=True)
                    nc.tensor.matmul(
                        pP1[:],
                        lhsT=woe_sb[:, c * W3:c * W3 + CIN],
                        rhs=xg[c][0][:],
                        start=(c == 0), stop=False, skip_group_check=True)
                # rank-1 weight row: -sum_ch gamma0[ch] Wo0[ch, ci]
                for c in range(2):
                    nc.tensor.matmul(pnw[:], lhsT=ngcol[:, c:c + 1],
                                     rhs=woe_sb[:, c * W3:c * W3 + CIN],
                                     start=(c == 0), stop=(c == 1))
                # beta @ Wo0 + bo (per-partition bias column for l=0)
                pbw = pstp.tile([CIN, 1], F32, tag="pbw", name="pbw")
                for c in range(2):
                    nc.tensor.matmul(pbw[:], lhsT=woe_sb[:, c * W3:c * W3 + CIN],
                                     rhs=bcol_sb[:, c:c + 1],
                                     start=(c == 0), stop=(c == 1))
                bo0 = workp.tile([CIN, 1], F32, tag="bo0", name="bo0")
                nc.vector.tensor_tensor(out=bo0[:], in0=pbw[:],
                                        in1=gb_sb[:, 6:7], op=ALU.add)

                # ---- scale rows: Act sqrt + DVE reciprocal
                sd1 = workp.tile([1, NR], F32, tag="sd1", name="sd1")
                sd2 = workp.tile([1, NR], F32, tag="sd2", name="sd2")
                nc.scalar.activation(sd1[:], pss1, AF.Sqrt,
                                     scale=1.0 / (3 * CH), bias=epsr[0:1, 0:1])
                nc.scalar.activation(sd2[:], pss2, AF.Sqrt,
                                     scale=1.0 / (5 * CH), bias=epsr[0:1, 0:1])
                rr1 = workp.tile([1, NR], BF16, tag="rr1", name="rr1")
                rr2 = workp.tile([1, NR], BF16, tag="rr2", name="rr2")
                with nc.allow_low_precision(reason="bf16 scales, as baseline"):
                    nc.vector.reciprocal(rr1[:], sd1[:])
                    nc.vector.reciprocal(rr2[:], sd2[:])
                # l=0 variance chain (DVE/Pool)
                mu_f = workp.tile([1, NR], F32, tag="mu_f", name="mu_f")
                nc.vector.tensor_scalar_mul(mu_f[:], pmu, 1.0 / CH)
                mrow = workp.tile([1, NR], BF16, tag="mrow", name="mrow")
                with nc.allow_low_precision(reason="bf16 scales, as baseline"):
                    nc.vector.tensor_scalar_mul(mrow[:], pmu, 1.0 / CH)
                m2 = workp.tile([1, NR], F32, tag="m2", name="m2")
                nc.vector.tensor_tensor(out=m2[:], in0=mu_f[:], in1=mu_f[:],
                                        op=ALU.mult)
                m2e = workp.tile([1, NR], F32, tag="m2e", name="m2e")
                nc.gpsimd.tensor_scalar(out=m2e[:], in0=m2[:], scalar1=1.0,
                                        scalar2=-EPS, op0=ALU.mult, op1=ALU.add)
                ve = workp.tile([1, NR], F32, tag="ve", name="ve")
                nc.vector.scalar_tensor_tensor(
                    out=ve[:], in0=pq1, scalar=1.0 / CH, in1=m2e[:],
                    op0=ALU.mult, op1=ALU.subtract)
                sd0 = workp.tile([1, NR], F32, tag="sd0", name="sd0")
                nc.scalar.activation(sd0[:], ve[:], AF.Sqrt)
                rstd = workp.tile([1, NR], BF16, tag="rstd", name="rstd")
                with nc.allow_low_precision(reason="bf16 scales, as baseline"):
                    nc.vector.reciprocal(rstd[:], sd0[:])
                nws0 = workp.tile([1, CIN], BF16, tag="nws0", name="nws0")
                nc.scalar.copy(nws0[:], pnw[:])

                # ---- broadcast scale rows to 128 partitions (K=1 matmuls)
                pA0 = pap.tile([128, NL2], F32, tag="pA0", name="pA0")
                pA1 = pap.tile([128, NR], F32, tag="pA1", name="pA1")
                for j in range(8):
                    row = rr1 if j < 3 else rr2
                    nc.tensor.matmul(pA0[:, j * NR:(j + 1) * NR],
                                     lhsT=ones1[:], rhs=row[:],
                                     start=True, stop=True,
                                     skip_group_check=True)
                nc.tensor.matmul(pA1[:], lhsT=ones1[:], rhs=rstd[:],
                                 start=True, stop=True)
                # mean rank-1 into the l=0 projection:  += (-sum g0 Wo0) x mu
                nc.tensor.matmul(pP1[:], lhsT=nws0[:], rhs=mrow[:],
                                 start=False, stop=True, skip_group_check=True)

                # ---- extract P to SBUF (Act+DVE), scale by the A rows
                psb0a = workp.tile([128, 256], BF16, tag="psb0a", name="psb0a")
                psb0b = workp.tile([128, 256], BF16, tag="psb0b", name="psb0b")
                nc.scalar.copy(psb0a[:], pP0[:, 0:256])
                nc.vector.tensor_copy(out=psb0b[:], in_=pP0[:, 256:NL2])
                psb1 = workp.tile([128, NR], BF16, tag="psb1", name="psb1")
                nc.scalar.copy(psb1[:], pP1[:])
                # l=0 first: its DMA leaves while y0 is still being scaled
                y1t = workp.tile([128, NR], F32, tag="y1t", name="y1t")
                nc.vector.tensor_tensor(out=y1t[:], in0=psb1[:], in1=pA1[:],
                                        op=ALU.mult)
                y1_sb = workp.tile([CIN, NR], BF16, tag="y1sb", name="y1sb")
                with nc.allow_low_precision(reason="bf16 out, host upcasts"):
                    nc.gpsimd.tensor_scalar_add(y1_sb[:], y1t[:], bo0[:, 0:1])
                nc.sync.dma_start(y1_d[:], y1_sb[:])
                y0_sb = workp.tile([CIN, NL2], BF16, tag="y0sb", name="y0sb")
                nc.vector.tensor_tensor(out=y0_sb[:, 0:256], in0=psb0a[:],
                                        in1=pA0[:, 0:256], op=ALU.mult)
                nc.vector.tensor_tensor(out=y0_sb[:, 256:NL2], in0=psb0b[:],
                                        in1=pA0[:, 256:NL2], op=ALU.mult)
                nc.sync.dma_start(y0_d[:], y0_sb[:])

                if debug_taps:
                    dbg_rows_d = nc.dram_tensor("dbg_rows", [1, 6 * NR], F32,
                                                kind="ExternalOutput")
                    dbg_psb_d = nc.dram_tensor("dbg_psb", [128, NL2 + NR], F32,
                                               kind="ExternalOutput")
                    dbg_pa_d = nc.dram_tensor("dbg_pa", [128, NL2 + NR], F32,
                                              kind="ExternalOutput")
                    drows = workp.tile([1, 6 * NR], F32, tag="drows", name="drows")
                    nc.vector.tensor_copy(out=drows[:, 0:NR], in_=rr1[:])
                    nc.vector.tensor_copy(out=drows[:, NR:2 * NR], in_=rr2[:])
                    nc.vector.tensor_copy(out=drows[:, 2 * NR:3 * NR], in_=rstd[:])
                    nc.vector.tensor_copy(out=drows[:, 3 * NR:4 * NR], in_=mu_f[:])
                    nc.vector.tensor_copy(out=drows[:, 4 * NR:5 * NR], in_=mrow[:])
                    nc.vector.tensor_copy(out=drows[:, 5 * NR:6 * NR], in_=ve[:])
                    nc.sync.dma_start(dbg_rows_d[:], drows[:])
                    dpsb = workp.tile([128, NL2 + NR], F32, tag="dpsb", name="dpsb")
                    nc.vector.tensor_copy(out=dpsb[:, 0:256], in_=psb0a[:])
                    nc.vector.tensor_copy(out=dpsb[:, 256:NL2], in_=psb0b[:])
                    nc.vector.tensor_copy(out=dpsb[:, NL2:], in_=psb1[:])
                    nc.sync.dma_start(dbg_psb_d[:], dpsb[:])
                    dpa = workp.tile([128, NL2 + NR], F32, tag="dpa", name="dpa")
                    nc.vector.tensor_copy(out=dpa[:, 0:NL2], in_=pA0[:])
                    nc.vector.tensor_copy(out=dpa[:, NL2:], in_=pA1[:])
                    nc.sync.dma_start(dbg_pa_d[:], dpa[:])

    _split_multiwaits(nc)
    return nc


# ------------------------------------------------------------------ host side
def _prep_inputs(inputs: dict[str, np.ndarray]):
    """Split the full inputs into per-core in_maps for the two phases
    (index bookkeeping and value re-layout only; all arithmetic on device)."""
    q = np.asarray(inputs["q"], np.float32).reshape(N, S, CIN)
    k = np.asarray(inputs["k"], np.float32).reshape(N, S, CIN)
    v = np.asarray(inputs["v"], np.float32).reshape(N, S, CIN)
    # host pre-transpose to [i, (s, m)] and cast to bf16; vT mt-major
    qT = np.ascontiguousarray(q.transpose(2, 1, 0).reshape(CIN, S * N)).astype(NP_BF16)
    kT = np.ascontiguousarray(k.transpose(2, 1, 0).reshape(CIN, S * N)).astype(NP_BF16)
    vT = np.ascontiguousarray(
        v.reshape(NT, 128, S, CIN).transpose(3, 0, 2, 1).reshape(CIN, S * N)
    ).astype(NP_BF16)
    env = np.asarray(inputs["envelope"], np.float32)
    attn_bias = np.asarray(inputs["attn_bias"], np.float32)
    a_idx = np.asarray(inputs["atom_index"]).astype(np.int64)
    b_idx = np.asarray(inputs["batch_index"]).astype(np.int64)
    e_map = np.asarray(inputs["edge_map_tab"]).astype(np.int64)
    Wq = np.asarray(inputs["Wq"], np.float32)
    Wk = np.asarray(inputs["Wk"], np.float32)
    Wv = np.asarray(inputs["Wv"], np.float32)
    bq = np.asarray(inputs["bq"], np.float32)
    bk = np.asarray(inputs["bk"], np.float32)
    bv = np.asarray(inputs["bv"], np.float32)
    gamma = np.asarray(inputs["gamma"], np.float32)
    beta = np.asarray(inputs["beta"], np.float32)
    Wo = np.asarray(inputs["Wo"], np.float32)
    bo = np.asarray(inputs["bo"], np.float32)

    # ---- slot layout for the (atom, segment) cells
    cell = a_idx * NSEG + b_idx                      # [E]
    order = np.argsort(cell, kind="stable")
    cell_s = cell[order]
    counts = np.bincount(cell_s, minlength=N * NSEG)
    L2 = int(counts.max())
    starts = np.zeros(N * NSEG, np.int64)
    starts[1:] = np.cumsum(counts)[:-1]
    rank = np.arange(E) - starts[cell_s]             # rank within cell
    m_s = cell_s // NSEG
    g_s = cell_s % NSEG
    p_s = m_s % 128
    t_s = m_s // 128
    col = (t_s * NSEG + g_s) * L2 + rank             # free-dim position
    Wd = NT * NSEG * L2
    env_e = env[e_map]                               # value gather (re-layout)
    envS = np.zeros((128, Wd), NP_BF16)
    envS[p_s, col] = env_e[order]
    bS_all = []
    for h in range(H):
        bs = np.zeros((128, Wd), NP_BF16)
        bs[p_s, col] = attn_bias[h, e_map][order]
        bS_all.append(bs)

    # ---- per-head weight slices, expanded per spherical component, [i,(s,o)]
    WqE = Wq[L_OF_M]                                 # [9, CIN, CH]
    WkE = Wk[L_OF_M]
    WvE = Wv[L_OF_M]

    in_maps1 = []
    for h in range(H):
        sl = slice(h * D, (h + 1) * D)
        in_maps1.append({
            "qT": qT, "kT": kT, "vT": vT,
            "wq": np.ascontiguousarray(
                WqE[:, :, sl].transpose(1, 0, 2).reshape(CIN, S * D)).astype(NP_BF16),
            "wk": np.ascontiguousarray(
                WkE[:, :, sl].transpose(1, 0, 2).reshape(CIN, S * D)).astype(NP_BF16),
            "wv": np.ascontiguousarray(
                WvE[:, :, sl].transpose(1, 0, 2).reshape(CIN, S * D)).astype(NP_BF16),
            "bqkv": np.ascontiguousarray(
                np.stack([bq[sl], bk[sl], bv[sl]], axis=1)
            ),
            "bvrow": np.ascontiguousarray(bv[sl].reshape(1, D)),
            "envs": envS,
            "bs": bS_all[h],
        })

    # ---- phase-2 constants
    # gb[p, 2l+c] = gamma[l, c*128+p];  gb[p, 6] = bo[p];  bcol[p, c] = beta
    gb = np.zeros((128, 7), np.float32)
    for l in range(LMAX + 1):
        for c in range(2):
            gb[:, 2 * l + c] = gamma[l, c * 128:(c + 1) * 128]
    gb[:, 6] = bo
    bcol = np.stack([beta[0:128], beta[128:256]], axis=1).astype(NP_BF16)
    # woe[p, c*384 + l*128 + ci] = Wo[l, c*128+p, ci]
    woe = np.zeros((128, 2 * (LMAX + 1) * CIN), NP_BF16)
    W3 = (LMAX + 1) * CIN
    for c in range(2):
        woe[:, c * W3:(c + 1) * W3] = Wo[:, c * 128:(c + 1) * 128, :].transpose(
            1, 0, 2).reshape(128, W3).astype(NP_BF16)
    p2_const = {"gb": gb, "bcol": bcol, "woe": woe}
    return in_maps1, L2, p2_const


def _reorder_ao(ao_all: list[np.ndarray]) -> list[np.ndarray]:
    """[h][N, (s,d)] -> per-core lnT [2, 128, (s_perm, n)] chunks, s-permuted
    channel-major slices for phase 2 (pure data movement)."""
    full = np.stack([np.asarray(a).reshape(N, S, D) for a in ao_all], axis=2)
    full = full.reshape(N, S, CH)                                 # [N, S, CH]
    out = []
    for c in range(H):
        xs = full[c * NR:(c + 1) * NR][:, SPERM, :]               # [64, 9, CH]
        t = xs.transpose(2, 1, 0).reshape(2, 128, SN)             # [CH, 9, 64]
        out.append(np.ascontiguousarray(t).astype(NP_BF16))
    return out


_BUILD_CACHE: dict = {}


def kernel(**inputs) -> np.ndarray:
    in_maps1, L2, p2_const = _prep_inputs(inputs)
    nc1 = _BUILD_CACHE.get(("p1", L2))
    if nc1 is None:
        nc1 = build_bass(L2)
        _BUILD_CACHE[("p1", L2)] = nc1
    res1 = run_bass_kernel_spmd(nc1, in_maps1, core_ids=list(range(H)))
    lnin_slices = _reorder_ao([r["ao"] for r in res1.results])

    nc2 = _BUILD_CACHE.get("p2")
    if nc2 is None:
        nc2 = build_phase2()
        _BUILD_CACHE["p2"] = nc2
    in_maps2 = [{"lnT": lnin_slices[c], **p2_const} for c in range(H)]
    res2 = run_bass_kernel_spmd(nc2, in_maps2, core_ids=list(range(H)))
    # y0T [ci, (s_perm[0:8], n)], y1T [ci, n] (bf16) -> y [N, S, CIN] f32
    y = np.zeros((N, S, CIN), np.float32)
    for c in range(H):
        yt = np.concatenate(
            [np.asarray(res2.results[c]["y0T"]),
             np.asarray(res2.results[c]["y1T"])], axis=1
        ).astype(np.float32).reshape(CIN, S, NR)
        y[c * NR:(c + 1) * NR][:, SPERM, :] = yt.transpose(2, 1, 0)
    return np.ascontiguousarray(y)

